# revision 12
# baseline (speedup 1.0000x reference)
"""Self-contained Trainium2 Bass kernel for nn_AttnBlock (VAE-style attention).

Reference computation (per batch b):
  hn = GroupNorm32(x)                      # [C, N], stats per group of 16 chans
  q/k/v = W @ hn + b                       # 1x1 convs, C=512
  attn = softmax(q^T k / sqrt(C), axis=j)  # N=4096 spatial positions
  out  = x + Wp @ (v @ attn^T) + bp

Sharding: 8 cores = 2 batches x 4 query chunks of 1024. Each core receives
its batch's full image ROLLED so its local 1024 query columns come first,
making the SPMD program identical on every core (key order under softmax is
permutation invariant). GroupNorm + K/V are computed over the full image on
each core; Q/proj/residual only for the local chunk.

Layout strategy (c = channel, j = key pos, i = query pos):
  hn   [c-part, n]     -> K tiles  [c-part, j]   (lhsT for scores)
                       -> vT tiles [j-part, c]   (lhsT for AV)
  scoresT = K^T Q      [j-part, i] via matmul(lhsT=k, rhs=q)
  E = exp(scoresT)     [j-part, i] (ScalarE, direct from PSUM)
  outU = vT^T @ E      [c-part, i] accumulated in PSUM over j
  den  = colsum(E)     via DVE accumulation + one all-ones matmul broadcast
  y = Wp @ (outU * recip_den) + (Wp@bv + bp) + x_local

All matmuls use float32r (full PE rate at free dim >= 256, near-fp32 precision).
"""

import numpy as np

import concourse.bass as bass
import concourse.mybir as mybir
from concourse import bacc
import concourse.tile as tile
from concourse import bass_utils

P = 128          # partitions
C = 512          # channels
CS = C // P      # channel slabs (4)
G = 32           # groups
GS = C // G      # channels per group (16)
EPS = 1e-6
F32 = mybir.dt.float32
F32R = mybir.dt.float32r
AL = mybir.AluOpType
AF = mybir.ActivationFunctionType
AX = mybir.AxisListType

N_FULL = 4096    # spatial positions (64*64)
NQ = 1024        # local query chunk per core
JT = 512         # j-tile (keys per outer iteration)
IT = 512         # i-tile (queries per scores matmul free dim)


def build_nc(n=N_FULL, nq=NQ):
    """Build the per-core Bass program. All 8 cores run this same program."""
    njt = n // JT
    nit = nq // IT

    nc = bacc.Bacc("TRN2", target_bir_lowering=False, debug=False)

    x_d = nc.dram_tensor("x", [C, n], F32, kind="ExternalInput")
    wq_d = nc.dram_tensor("wq_t", [C, C], F32, kind="ExternalInput")  # [c_in, c_out]
    wk_d = nc.dram_tensor("wk_t", [C, C], F32, kind="ExternalInput")
    wv_d = nc.dram_tensor("wv_t", [C, C], F32, kind="ExternalInput")
    wp_d = nc.dram_tensor("wp_t", [C, C], F32, kind="ExternalInput")
    bq_d = nc.dram_tensor("bq", [C], F32, kind="ExternalInput")    # pre-scaled
    bk_d = nc.dram_tensor("bk", [C], F32, kind="ExternalInput")
    bp_d = nc.dram_tensor("bp_eff", [C], F32, kind="ExternalInput")  # Wp@bv + bp
    gam_d = nc.dram_tensor("gamma", [C], F32, kind="ExternalInput")
    bet_d = nc.dram_tensor("beta", [C], F32, kind="ExternalInput")
    bo_d = nc.dram_tensor("bo", [P, P], F32, kind="ExternalInput")   # blockdiag 1/16
    on_d = nc.dram_tensor("ones", [P, P], F32, kind="ExternalInput")
    y_d = nc.dram_tensor("y", [C, nq], F32, kind="ExternalOutput")

    # [C, n] viewed as [P, CS, n]: channel c = slab*128 + partition
    x_t = x_d.rearrange("(o p) n -> p o n", p=P)
    y_t = y_d.rearrange("(o p) n -> p o n", p=P)

    def r2(d):  # [C] -> [P, CS]
        return d.rearrange("(o p) -> p o", p=P)

    def rw(d):  # [C, C] -> [P, CS, C]  (contraction dim on partitions)
        return d.rearrange("(o p) c -> p o c", p=P)

    with tile.TileContext(nc) as tc:
        with (
            tc.tile_pool(name="consts", bufs=1) as consts,
            tc.tile_pool(name="xp", bufs=1) as xp,
            tc.tile_pool(name="persist", bufs=1) as persist,
            tc.tile_pool(name="kv", bufs=2) as kv,
            tc.tile_pool(name="ep", bufs=3) as ep,
            tc.tile_pool(name="xres", bufs=3) as xres,
            tc.tile_pool(name="yp", bufs=2) as yp,
            tc.tile_pool(name="psmm", bufs=2, space="PSUM") as psmm,
            tc.tile_pool(name="psacc", bufs=4, space="PSUM") as psacc,
        ):
            # ---- constants ----
            wq_sb = consts.tile([P, CS, C], F32R, tag="wq")
            wk_sb = consts.tile([P, CS, C], F32R, tag="wk")
            wv_sb = consts.tile([P, CS, C], F32R, tag="wv")
            wp_sb = consts.tile([P, CS, C], F32R, tag="wp")
            nc.sync.dma_start(out=wq_sb, in_=rw(wq_d).bitcast(F32R))
            nc.sync.dma_start(out=wk_sb, in_=rw(wk_d).bitcast(F32R))
            nc.sync.dma_start(out=wv_sb, in_=rw(wv_d).bitcast(F32R))
            nc.sync.dma_start(out=wp_sb, in_=rw(wp_d).bitcast(F32R))
            bq_sb = consts.tile([P, CS], F32, tag="bq")
            bk_sb = consts.tile([P, CS], F32, tag="bk")
            bp_sb = consts.tile([P, CS], F32, tag="bp")
            gam_sb = consts.tile([P, CS], F32, tag="gam")
            bet_sb = consts.tile([P, CS], F32, tag="bet")
            nc.sync.dma_start(out=bq_sb, in_=r2(bq_d))
            nc.sync.dma_start(out=bk_sb, in_=r2(bk_d))
            nc.sync.dma_start(out=bp_sb, in_=r2(bp_d))
            nc.sync.dma_start(out=gam_sb, in_=r2(gam_d))
            nc.sync.dma_start(out=bet_sb, in_=r2(bet_d))
            bo_sb = consts.tile([P, P], F32, tag="bo")
            on_sb = consts.tile([P, P], F32, tag="on")
            nc.sync.dma_start(out=bo_sb, in_=bo_d[:, :])
            nc.sync.dma_start(out=on_sb, in_=on_d[:, :])
            eps_sb = consts.tile([P, 1], F32, tag="eps")
            nc.vector.memset(eps_sb, EPS)

            # ---- phase 1: load x, groupnorm stats, normalize in place ----
            x_sbs = []
            mv = consts.tile([P, CS, 2], F32, tag="mv")  # per-part mean, var->Ex2
            for po in range(CS):
                xs = xp.tile([P, n], F32, tag=f"x{po}")
                nc.sync.dma_start(out=xs.bitcast(F32R),
                                  in_=x_t[:, po, :].bitcast(F32R))
                x_sbs.append(xs)
                nchunk = n // 512
                stats = consts.tile([P, nchunk, 6], F32, tag=f"st{po}")
                xs3 = xs.rearrange("p (s f) -> p s f", f=512)
                for s in range(nchunk):
                    nc.vector.bn_stats(out=stats[:, s, :], in_=xs3[:, s, :])
                nc.vector.bn_aggr(out=mv[:, po, :], in_=stats)
                # var -> E[x^2] = mean*mean + var (in place)
                nc.vector.scalar_tensor_tensor(
                    out=mv[:, po, 1:2], in0=mv[:, po, 0:1],
                    scalar=mv[:, po, 0:1],
                    in1=mv[:, po, 1:2], op0=AL.mult, op1=AL.add)

            # group-average per channel: [P, CS*2] = BO^T @ mv
            ps_st = psmm.tile([P, CS * 2], F32, tag="ps_mm")
            nc.tensor.matmul(ps_st, bo_sb,
                             mv.rearrange("p a b -> p (a b)"),
                             start=True, stop=True)
            mvg = consts.tile([P, CS * 2], F32, tag="mvg")
            nc.vector.tensor_copy(out=mvg, in_=ps_st)
            ps3 = mvg.rearrange("p (a b) -> p a b", b=2)
            gmean = ps3[:, :, 0]   # group E[x] per channel
            gex2 = ps3[:, :, 1]    # group E[x^2] per channel
            tmp = consts.tile([P, CS], F32, tag="tmp")
            var = consts.tile([P, CS], F32, tag="var")
            rstd = consts.tile([P, CS], F32, tag="rstd")
            scl = consts.tile([P, CS], F32, tag="scl")
            shf = consts.tile([P, CS], F32, tag="shf")
            nc.vector.tensor_tensor(out=tmp, in0=gmean, in1=gmean, op=AL.mult)
            nc.vector.tensor_sub(out=var, in0=gex2, in1=tmp)
            nc.scalar.activation(out=var, in_=var, func=AF.Sqrt, bias=eps_sb,
                                 scale=1.0)
            nc.vector.reciprocal(out=rstd, in_=var)
            nc.vector.tensor_tensor(out=scl, in0=rstd, in1=gam_sb, op=AL.mult)
            nc.vector.tensor_tensor(out=tmp, in0=gmean, in1=scl, op=AL.mult)
            nc.vector.tensor_sub(out=shf, in0=bet_sb, in1=tmp)
            for po in range(CS):
                nc.vector.tensor_scalar(
                    out=x_sbs[po].bitcast(F32R), in0=x_sbs[po],
                    scalar1=scl[:, po:po + 1], scalar2=shf[:, po:po + 1],
                    op0=AL.mult, op1=AL.add)

            # ---- phase 2: q = Wq_s @ hn + bq_s for local queries ----
            q_sb = persist.tile([P, CS, nq], F32, tag="q")
            for cc in range(CS):
                for it in range(nit):
                    isl = slice(it * IT, (it + 1) * IT)
                    ps = psmm.tile([P, IT], F32, tag="ps_mm")
                    for ks in range(CS):
                        nc.tensor.matmul(
                            ps,
                            wq_sb[:, ks, cc * P:(cc + 1) * P].bitcast(F32R),
                            x_sbs[ks][:, 0:nq][:, isl].bitcast(F32R),
                            start=(ks == 0), stop=(ks == CS - 1))
                    nc.vector.tensor_scalar_add(
                        out=q_sb[:, cc, isl].bitcast(F32R), in0=ps,
                        scalar1=bq_sb[:, cc:cc + 1])

            # ---- phase 3: stream over j-tiles ----
            outu = persist.tile([P, CS, nq], F32, tag="outu")
            dens = [persist.tile([P, IT], F32, tag=f"den{it}", name=f"den{it}")
                    for it in range(nit)]

            for jt in range(njt):
                jsl = slice(jt * JT, (jt + 1) * JT)
                # k tile [c-part, j]: 16 matmuls
                k_sb = kv.tile([P, CS, JT], F32, tag="k")
                for cc in range(CS):
                    ps = psmm.tile([P, JT], F32, tag="ps_mm")
                    for ks in range(CS):
                        nc.tensor.matmul(
                            ps,
                            wk_sb[:, ks, cc * P:(cc + 1) * P].bitcast(F32R),
                            x_sbs[ks][:, jsl].bitcast(F32R),
                            start=(ks == 0), stop=(ks == CS - 1))
                    nc.vector.tensor_scalar_add(
                        out=k_sb[:, cc, :].bitcast(F32R), in0=ps,
                        scalar1=bk_sb[:, cc:cc + 1])
                # vT tile [j-part, c]: 16 matmuls (hn as lhsT; v bias folded into bp_eff)
                vt_sb = kv.tile([P, JT // P, C], F32, tag="vt")
                for jc in range(JT // P):
                    jcs = slice(jt * JT + jc * P, jt * JT + (jc + 1) * P)
                    ps = psmm.tile([P, C], F32, tag="ps_mm")
                    for ks in range(CS):
                        nc.tensor.matmul(
                            ps,
                            x_sbs[ks][:, jcs].bitcast(F32R),
                            wv_sb[:, ks, :].bitcast(F32R),
                            start=(ks == 0), stop=(ks == CS - 1))
                    nc.vector.tensor_copy(out=vt_sb[:, jc, :].bitcast(F32R), in_=ps)

                for it in range(nit):
                    isl = slice(it * IT, (it + 1) * IT)
                    pos = [psacc.tile([P, IT], F32, tag="po", name=f"po{cc}")
                           for cc in range(CS)]
                    for jc in range(JT // P):
                        # scoresT chunk [j-part 128, i 512]
                        ps_s = psmm.tile([P, IT], F32, tag="ps_mm")
                        for ks in range(CS):
                            nc.tensor.matmul(
                                ps_s,
                                k_sb[:, ks, jc * P:(jc + 1) * P].bitcast(F32R),
                                q_sb[:, ks, isl].bitcast(F32R),
                                start=(ks == 0), stop=(ks == CS - 1))
                        e_sb = ep.tile([P, IT], F32, tag="e")
                        nc.scalar.activation(out=e_sb.bitcast(F32R), in_=ps_s,
                                             func=AF.Exp, scale=1.0)
                        # denominator partials (per j-partition)
                        if jt == 0 and jc == 0:
                            nc.vector.tensor_copy(out=dens[it], in_=e_sb)
                        else:
                            nc.vector.tensor_add(out=dens[it], in0=dens[it],
                                                 in1=e_sb)
                        # AV accumulate into psum over this j-tile
                        for cc in range(CS):
                            nc.tensor.matmul(
                                pos[cc],
                                vt_sb[:, jc, cc * P:(cc + 1) * P].bitcast(F32R),
                                e_sb.bitcast(F32R),
                                start=(jc == 0), stop=(jc == JT // P - 1))
                    # drain psum accumulators into SBUF outU
                    for cc in range(CS):
                        if jt == 0:
                            nc.vector.tensor_copy(out=outu[:, cc, isl], in_=pos[cc])
                        else:
                            nc.vector.tensor_add(out=outu[:, cc, isl],
                                                 in0=outu[:, cc, isl], in1=pos[cc])

            # ---- phase 4: normalize, proj, residual ----
            recip = persist.tile([P, nq], F32, tag="recip")
            for it in range(nit):
                isl = slice(it * IT, (it + 1) * IT)
                ps_d = psmm.tile([P, IT], F32, tag="ps_mm")
                nc.tensor.matmul(ps_d, on_sb, dens[it],
                                 start=True, stop=True)
                nc.vector.reciprocal(out=recip[:, isl], in_=ps_d)
            for cc in range(CS):
                for it in range(nit):
                    isl = slice(it * IT, (it + 1) * IT)
                    # reuse q_sb (dead after attention) as the normalized buffer
                    nc.vector.tensor_tensor(out=q_sb[:, cc, isl].bitcast(F32R),
                                            in0=outu[:, cc, isl],
                                            in1=recip[:, isl], op=AL.mult)
            for cc in range(CS):
                for it in range(nit):
                    isl = slice(it * IT, (it + 1) * IT)
                    ps = psmm.tile([P, IT], F32, tag="ps_mm")
                    for ks in range(CS):
                        nc.tensor.matmul(
                            ps,
                            wp_sb[:, ks, cc * P:(cc + 1) * P].bitcast(F32R),
                            q_sb[:, ks, isl].bitcast(F32R),
                            start=(ks == 0), stop=(ks == CS - 1))
                    xr = xres.tile([P, IT], F32, tag="xr")
                    nc.sync.dma_start(out=xr, in_=x_t[:, cc, isl])
                    yt = yp.tile([P, IT], F32, tag="yt")
                    nc.vector.scalar_tensor_tensor(
                        out=yt, in0=ps, scalar=bp_sb[:, cc:cc + 1], in1=xr,
                        op0=AL.add, op1=AL.add)
                    nc.sync.dma_start(out=y_t[:, cc, isl], in_=yt)

    nc.compile()
    return nc


_NC_CACHE = {}


def _get_nc(n=N_FULL, nq=NQ):
    key = (n, nq)
    if key not in _NC_CACHE:
        _NC_CACHE[key] = build_nc(n, nq)
    return _NC_CACHE[key]


def make_in_maps(x, q_w, q_b, k_w, k_b, v_w, v_b, proj_w, proj_b,
                 norm_gamma, norm_beta, n_cores=8):
    """Build per-core input dicts from the full problem inputs."""
    B = x.shape[0]
    n = x.shape[2] * x.shape[3]
    xf = np.ascontiguousarray(x.reshape(B, C, n).astype(np.float32))
    scale = np.float32(C) ** np.float32(-0.5)
    wq_t = np.ascontiguousarray((q_w * scale).T.astype(np.float32))
    wk_t = np.ascontiguousarray(k_w.T.astype(np.float32))
    wv_t = np.ascontiguousarray(v_w.T.astype(np.float32))
    wp_t = np.ascontiguousarray(proj_w.T.astype(np.float32))
    bq = (q_b * scale).astype(np.float32)
    bp_eff = (proj_w.astype(np.float64) @ v_b.astype(np.float64)
              + proj_b.astype(np.float64)).astype(np.float32)
    # block-diagonal group-averaging matrix: 16x16 blocks of 1/16
    bo = np.zeros((P, P), np.float32)
    for g in range(P // GS):
        bo[g * GS:(g + 1) * GS, g * GS:(g + 1) * GS] = 1.0 / GS
    ones = np.ones((P, P), np.float32)
    chunks = n_cores // B
    nq = n // chunks
    in_maps = []
    for g in range(n_cores):
        b, qc = divmod(g, chunks)
        xg = np.roll(xf[b], -qc * nq, axis=1)
        in_maps.append(dict(
            x=np.ascontiguousarray(xg), wq_t=wq_t, wk_t=wk_t, wv_t=wv_t,
            wp_t=wp_t, bq=bq, bk=k_b.astype(np.float32), bp_eff=bp_eff,
            gamma=norm_gamma.astype(np.float32), beta=norm_beta.astype(np.float32),
            bo=bo, ones=ones))
    return in_maps


def kernel(**inputs):
    x = np.asarray(inputs["x"], np.float32)
    B, _, H, W = x.shape
    n = H * W
    chunks = 8 // B
    nq = n // chunks
    in_maps = make_in_maps(
        x, np.asarray(inputs["q_w"]), np.asarray(inputs["q_b"]),
        np.asarray(inputs["k_w"]), np.asarray(inputs["k_b"]),
        np.asarray(inputs["v_w"]), np.asarray(inputs["v_b"]),
        np.asarray(inputs["proj_w"]), np.asarray(inputs["proj_b"]),
        np.asarray(inputs["norm_gamma"]), np.asarray(inputs["norm_beta"]))
    nc = _get_nc(n, nq)
    res = bass_utils.run_bass_kernel_spmd(nc, in_maps, core_ids=list(range(8)))
    y = np.empty((B, C, n), np.float32)
    for g in range(8):
        b, qc = divmod(g, chunks)
        y[b][:, qc * nq:(qc + 1) * nq] = res.results[g]["y"]
    return y.reshape(B, C, H, W)


# revision 13
# speedup vs baseline: 88.3989x; 88.3989x over previous
"""Self-contained Trainium2 Bass kernel for nn_AttnBlock (VAE-style attention).

Reference computation (per batch b):
  hn = GroupNorm32(x)                      # [C, N], stats per group of 16 chans
  q/k/v = W @ hn + b                       # 1x1 convs, C=512
  attn = softmax(q^T k / sqrt(C), axis=j)  # N=4096 spatial positions
  out  = x + Wp @ (v @ attn^T) + bp

Sharding: 8 cores = 2 batches x 4 query chunks of 1024. Each core receives
its batch's full image ROLLED so its local 1024 query columns come first,
making the SPMD program identical on every core (key order under softmax is
permutation invariant). GroupNorm + K/V are computed over the full image on
each core; Q/proj/residual only for the local chunk.

Layout strategy (c = channel, j = key pos, i = query pos):
  hn   [c-part, n]     -> K tiles  [c-part, j]   (lhsT for scores)
                       -> vT tiles [j-part, c]   (lhsT for AV)
  scoresT = K^T Q      [j-part, i] via matmul(lhsT=k, rhs=q)
  E = exp(scoresT)     [j-part, i] (ScalarE, direct from PSUM)
  outU = vT^T @ E      [c-part, i] accumulated in PSUM over j
  den  = colsum(E)     via DVE accumulation + one all-ones matmul broadcast
  y = Wp @ (outU * recip_den) + (Wp@bv + bp) + x_local

All big matmuls use float32r (full PE rate at free dim >= 256, ~fp32 precision).
"""

import numpy as np

import concourse.bass as bass
import concourse.mybir as mybir
from concourse import bacc
import concourse.tile as tile
from concourse import bass_utils

P = 128          # partitions
C = 512          # channels
CS = C // P      # channel slabs (4)
G = 32           # groups
GS = C // G      # channels per group (16)
EPS = 1e-6
F32 = mybir.dt.float32
F32R = mybir.dt.float32r
AL = mybir.AluOpType
AF = mybir.ActivationFunctionType

N_FULL = 4096    # spatial positions (64*64)
NQ = 1024        # local query chunk per core
JT = 512         # j-tile (keys per outer iteration)
IT = 512         # i-tile (queries per scores matmul free dim)


def build_nc(n=N_FULL, nq=NQ, repeat=1):
    """Build the per-core Bass program. All 8 cores run this same program.

    repeat > 1 wraps the whole compute body in a hardware loop — used only
    for benchmarking (amortizes host dispatch overhead for timing).
    """
    njt = n // JT
    nit = nq // IT

    nc = bacc.Bacc("TRN2", target_bir_lowering=False, debug=False)

    x_d = nc.dram_tensor("x", [C, n], F32, kind="ExternalInput")
    wq_d = nc.dram_tensor("wq_t", [C, C], F32, kind="ExternalInput")  # [cin, cout]
    wk_d = nc.dram_tensor("wk_t", [C, C], F32, kind="ExternalInput")
    wv_d = nc.dram_tensor("wv_t", [C, C], F32, kind="ExternalInput")
    wp_d = nc.dram_tensor("wp_t", [C, C], F32, kind="ExternalInput")
    bq_d = nc.dram_tensor("bq", [C], F32, kind="ExternalInput")    # pre-scaled
    bk_d = nc.dram_tensor("bk", [C], F32, kind="ExternalInput")
    bp_d = nc.dram_tensor("bp_eff", [C], F32, kind="ExternalInput")  # Wp@bv + bp
    gam_d = nc.dram_tensor("gamma", [C], F32, kind="ExternalInput")
    bet_d = nc.dram_tensor("beta", [C], F32, kind="ExternalInput")
    bo_d = nc.dram_tensor("bo", [P, P], F32, kind="ExternalInput")   # blockdiag 1/16
    on_d = nc.dram_tensor("ones", [P, P], F32, kind="ExternalInput")
    y_d = nc.dram_tensor("y", [C, nq], F32, kind="ExternalOutput")

    # [C, n] viewed as [P, CS, n]: channel c = slab*128 + partition
    x_t = x_d.rearrange("(o p) n -> p o n", p=P)
    y_t = y_d.rearrange("(o p) n -> p o n", p=P)

    def r2(d):  # [C] -> [P, CS]
        return d.rearrange("(o p) -> p o", p=P)

    def rw(d):  # [C, C] -> [P, CS, C]  (contraction dim on partitions)
        return d.rearrange("(o p) c -> p o c", p=P)

    with tile.TileContext(nc) as tc:
        with (
            tc.tile_pool(name="consts", bufs=1) as consts,
            tc.tile_pool(name="xp", bufs=1) as xp,
            tc.tile_pool(name="persist", bufs=1) as persist,
            tc.tile_pool(name="kv", bufs=2) as kv,
            tc.tile_pool(name="ep", bufs=3) as ep,
            tc.tile_pool(name="xres", bufs=3) as xres,
            tc.tile_pool(name="yp", bufs=2) as yp,
            tc.tile_pool(name="psmm", bufs=2, space="PSUM") as psmm,
            tc.tile_pool(name="psacc", bufs=4, space="PSUM") as psacc,
        ):
            # ---- constants (outside the repeat loop) ----
            wq_sb = consts.tile([P, CS, C], F32R, tag="wq")
            wk_sb = consts.tile([P, CS, C], F32R, tag="wk")
            wv_sb = consts.tile([P, CS, C], F32R, tag="wv")
            wp_sb = consts.tile([P, CS, C], F32R, tag="wp")
            nc.sync.dma_start(out=wq_sb, in_=rw(wq_d).bitcast(F32R))
            nc.sync.dma_start(out=wk_sb, in_=rw(wk_d).bitcast(F32R))
            nc.sync.dma_start(out=wv_sb, in_=rw(wv_d).bitcast(F32R))
            nc.sync.dma_start(out=wp_sb, in_=rw(wp_d).bitcast(F32R))
            bq_sb = consts.tile([P, CS], F32, tag="bq")
            bk_sb = consts.tile([P, CS], F32, tag="bk")
            bp_sb = consts.tile([P, CS], F32, tag="bp")
            gam_sb = consts.tile([P, CS], F32, tag="gam")
            bet_sb = consts.tile([P, CS], F32, tag="bet")
            nc.sync.dma_start(out=bq_sb, in_=r2(bq_d))
            nc.sync.dma_start(out=bk_sb, in_=r2(bk_d))
            nc.sync.dma_start(out=bp_sb, in_=r2(bp_d))
            nc.sync.dma_start(out=gam_sb, in_=r2(gam_d))
            nc.sync.dma_start(out=bet_sb, in_=r2(bet_d))
            bo_sb = consts.tile([P, P], F32, tag="bo")
            on_sb = consts.tile([P, P], F32, tag="on")
            nc.sync.dma_start(out=bo_sb, in_=bo_d[:, :])
            nc.sync.dma_start(out=on_sb, in_=on_d[:, :])
            eps_sb = consts.tile([P, 1], F32, tag="eps")
            nc.vector.memset(eps_sb, EPS)

            def body():
                # ---- phase 1: load x, groupnorm stats, normalize in place ----
                x_sbs = []
                mv = consts.tile([P, CS, 2], F32, tag="mv", name="mv")
                for po in range(CS):
                    xs = xp.tile([P, n], F32, tag=f"x{po}", name=f"x{po}")
                    nc.sync.dma_start(out=xs.bitcast(F32R),
                                      in_=x_t[:, po, :].bitcast(F32R))
                    x_sbs.append(xs)
                    nchunk = n // 512
                    stats = consts.tile([P, nchunk, 6], F32, tag=f"st{po}",
                                        name=f"st{po}")
                    xs3 = xs.rearrange("p (s f) -> p s f", f=512)
                    for s in range(nchunk):
                        nc.vector.bn_stats(out=stats[:, s, :], in_=xs3[:, s, :])
                    nc.vector.bn_aggr(out=mv[:, po, :], in_=stats)
                    # var -> E[x^2] = mean*mean + var (in place)
                    nc.vector.scalar_tensor_tensor(
                        out=mv[:, po, 1:2], in0=mv[:, po, 0:1],
                        scalar=mv[:, po, 0:1],
                        in1=mv[:, po, 1:2], op0=AL.mult, op1=AL.add)

                # group-average per channel: [P, CS*2] = BO^T @ mv
                ps_st = psmm.tile([P, CS * 2], F32, tag="ps_mm", name="ps_st")
                nc.tensor.matmul(ps_st, bo_sb,
                                 mv.rearrange("p a b -> p (a b)"),
                                 start=True, stop=True)
                mvg = consts.tile([P, CS * 2], F32, tag="mvg", name="mvg")
                nc.vector.tensor_copy(out=mvg, in_=ps_st)
                ps3 = mvg.rearrange("p (a b) -> p a b", b=2)
                gmean = ps3[:, :, 0]   # group E[x] per channel
                gex2 = ps3[:, :, 1]    # group E[x^2] per channel
                tmp = consts.tile([P, CS], F32, tag="tmp", name="tmp")
                var = consts.tile([P, CS], F32, tag="var", name="var")
                rstd = consts.tile([P, CS], F32, tag="rstd", name="rstd")
                scl = consts.tile([P, CS], F32, tag="scl", name="scl")
                shf = consts.tile([P, CS], F32, tag="shf", name="shf")
                nc.vector.tensor_tensor(out=tmp, in0=gmean, in1=gmean, op=AL.mult)
                nc.vector.tensor_sub(out=var, in0=gex2, in1=tmp)
                nc.scalar.activation(out=var, in_=var, func=AF.Sqrt, bias=eps_sb,
                                     scale=1.0)
                nc.vector.reciprocal(out=rstd, in_=var)
                nc.vector.tensor_tensor(out=scl, in0=rstd, in1=gam_sb, op=AL.mult)
                nc.vector.tensor_tensor(out=tmp, in0=gmean, in1=scl, op=AL.mult)
                nc.vector.tensor_sub(out=shf, in0=bet_sb, in1=tmp)
                for po in range(CS):
                    nc.vector.tensor_scalar(
                        out=x_sbs[po].bitcast(F32R), in0=x_sbs[po],
                        scalar1=scl[:, po:po + 1], scalar2=shf[:, po:po + 1],
                        op0=AL.mult, op1=AL.add)

                # ---- phase 2: q = Wq_s @ hn + bq_s for local queries ----
                q_sb = persist.tile([P, CS, nq], F32, tag="q", name="q")
                for cc in range(CS):
                    for it in range(nit):
                        isl = slice(it * IT, (it + 1) * IT)
                        ps = psmm.tile([P, IT], F32, tag="ps_mm", name="ps_q")
                        for ks in range(CS):
                            nc.tensor.matmul(
                                ps,
                                wq_sb[:, ks, cc * P:(cc + 1) * P].bitcast(F32R),
                                x_sbs[ks][:, 0:nq][:, isl].bitcast(F32R),
                                start=(ks == 0), stop=(ks == CS - 1))
                        nc.vector.tensor_scalar_add(
                            out=q_sb[:, cc, isl].bitcast(F32R), in0=ps,
                            scalar1=bq_sb[:, cc:cc + 1])

                # ---- phase 3: stream over j-tiles ----
                outu = persist.tile([P, CS, nq], F32, tag="outu", name="outu")
                dens = [persist.tile([P, IT], F32, tag=f"den{it}", name=f"den{it}")
                        for it in range(nit)]

                for jt in range(njt):
                    jsl = slice(jt * JT, (jt + 1) * JT)
                    # k tile [c-part, j]: 16 matmuls
                    k_sb = kv.tile([P, CS, JT], F32, tag="k", name="k")
                    for cc in range(CS):
                        ps = psmm.tile([P, JT], F32, tag="ps_mm", name="ps_k")
                        for ks in range(CS):
                            nc.tensor.matmul(
                                ps,
                                wk_sb[:, ks, cc * P:(cc + 1) * P].bitcast(F32R),
                                x_sbs[ks][:, jsl].bitcast(F32R),
                                start=(ks == 0), stop=(ks == CS - 1))
                        nc.vector.tensor_scalar_add(
                            out=k_sb[:, cc, :].bitcast(F32R), in0=ps,
                            scalar1=bk_sb[:, cc:cc + 1])
                    # vT tile [j-part, c]: 16 matmuls (v bias folded into bp_eff)
                    vt_sb = kv.tile([P, JT // P, C], F32, tag="vt", name="vt")
                    for jc in range(JT // P):
                        jcs = slice(jt * JT + jc * P, jt * JT + (jc + 1) * P)
                        ps = psmm.tile([P, C], F32, tag="ps_mm", name="ps_v")
                        for ks in range(CS):
                            nc.tensor.matmul(
                                ps,
                                x_sbs[ks][:, jcs].bitcast(F32R),
                                wv_sb[:, ks, :].bitcast(F32R),
                                start=(ks == 0), stop=(ks == CS - 1))
                        nc.vector.tensor_copy(out=vt_sb[:, jc, :].bitcast(F32R),
                                              in_=ps)

                    for it in range(nit):
                        isl = slice(it * IT, (it + 1) * IT)
                        pos = [psacc.tile([P, IT], F32, tag="po", name=f"po{cc}")
                               for cc in range(CS)]
                        for jc in range(JT // P):
                            # scoresT chunk [j-part 128, i 512]
                            ps_s = psmm.tile([P, IT], F32, tag="ps_mm", name="ps_s")
                            for ks in range(CS):
                                nc.tensor.matmul(
                                    ps_s,
                                    k_sb[:, ks, jc * P:(jc + 1) * P].bitcast(F32R),
                                    q_sb[:, ks, isl].bitcast(F32R),
                                    start=(ks == 0), stop=(ks == CS - 1))
                            e_sb = ep.tile([P, IT], F32, tag="e", name="e")
                            nc.scalar.activation(out=e_sb.bitcast(F32R), in_=ps_s,
                                                 func=AF.Exp, scale=1.0)
                            # denominator partials (per j-partition)
                            if jt == 0 and jc == 0:
                                nc.vector.tensor_copy(out=dens[it], in_=e_sb)
                            else:
                                nc.vector.tensor_add(out=dens[it], in0=dens[it],
                                                     in1=e_sb)
                            # AV accumulate into psum over this j-tile
                            for cc in range(CS):
                                nc.tensor.matmul(
                                    pos[cc],
                                    vt_sb[:, jc, cc * P:(cc + 1) * P].bitcast(F32R),
                                    e_sb.bitcast(F32R),
                                    start=(jc == 0), stop=(jc == JT // P - 1))
                        # drain psum accumulators into SBUF outU
                        for cc in range(CS):
                            if jt == 0:
                                nc.vector.tensor_copy(out=outu[:, cc, isl],
                                                      in_=pos[cc])
                            else:
                                nc.vector.tensor_add(out=outu[:, cc, isl],
                                                     in0=outu[:, cc, isl],
                                                     in1=pos[cc])

                # ---- phase 4: normalize, proj, residual ----
                recip = persist.tile([P, nq], F32, tag="recip", name="recip")
                for it in range(nit):
                    isl = slice(it * IT, (it + 1) * IT)
                    ps_d = psmm.tile([P, IT], F32, tag="ps_mm", name="ps_d")
                    nc.tensor.matmul(ps_d, on_sb, dens[it], start=True, stop=True)
                    nc.vector.reciprocal(out=recip[:, isl], in_=ps_d)
                for cc in range(CS):
                    for it in range(nit):
                        isl = slice(it * IT, (it + 1) * IT)
                        # reuse q_sb (dead after attention) as normalized buffer
                        nc.vector.tensor_tensor(
                            out=q_sb[:, cc, isl].bitcast(F32R),
                            in0=outu[:, cc, isl],
                            in1=recip[:, isl], op=AL.mult)
                for cc in range(CS):
                    for it in range(nit):
                        isl = slice(it * IT, (it + 1) * IT)
                        ps = psmm.tile([P, IT], F32, tag="ps_mm", name="ps_p")
                        for ks in range(CS):
                            nc.tensor.matmul(
                                ps,
                                wp_sb[:, ks, cc * P:(cc + 1) * P].bitcast(F32R),
                                q_sb[:, ks, isl].bitcast(F32R),
                                start=(ks == 0), stop=(ks == CS - 1))
                        xr = xres.tile([P, IT], F32, tag="xr", name="xr")
                        nc.sync.dma_start(out=xr, in_=x_t[:, cc, isl])
                        yt = yp.tile([P, IT], F32, tag="yt", name="yt")
                        nc.vector.scalar_tensor_tensor(
                            out=yt, in0=ps, scalar=bp_sb[:, cc:cc + 1], in1=xr,
                            op0=AL.add, op1=AL.add)
                        nc.sync.dma_start(out=y_t[:, cc, isl], in_=yt)

            if repeat == 1:
                body()
            else:
                with tc.For_i(0, repeat, 1):
                    body()

    nc.compile()
    return nc


_NC_CACHE = {}


def _get_nc(n=N_FULL, nq=NQ, repeat=1):
    key = (n, nq, repeat)
    if key not in _NC_CACHE:
        _NC_CACHE[key] = build_nc(n, nq, repeat)
    return _NC_CACHE[key]


def make_in_maps(x, q_w, q_b, k_w, k_b, v_w, v_b, proj_w, proj_b,
                 norm_gamma, norm_beta, n_cores=8):
    """Build per-core input dicts from the full problem inputs."""
    B = x.shape[0]
    n = x.shape[2] * x.shape[3]
    xf = np.ascontiguousarray(x.reshape(B, C, n).astype(np.float32))
    scale = np.float32(C) ** np.float32(-0.5)
    wq_t = np.ascontiguousarray((q_w * scale).T.astype(np.float32))
    wk_t = np.ascontiguousarray(k_w.T.astype(np.float32))
    wv_t = np.ascontiguousarray(v_w.T.astype(np.float32))
    wp_t = np.ascontiguousarray(proj_w.T.astype(np.float32))
    bq = (q_b * scale).astype(np.float32)
    bp_eff = (proj_w.astype(np.float64) @ v_b.astype(np.float64)
              + proj_b.astype(np.float64)).astype(np.float32)
    # block-diagonal group-averaging matrix: 16x16 blocks of 1/16
    bo = np.zeros((P, P), np.float32)
    for g in range(P // GS):
        bo[g * GS:(g + 1) * GS, g * GS:(g + 1) * GS] = 1.0 / GS
    ones = np.ones((P, P), np.float32)
    chunks = n_cores // B
    nq = n // chunks
    in_maps = []
    for g in range(n_cores):
        b, qc = divmod(g, chunks)
        xg = np.roll(xf[b], -qc * nq, axis=1)
        in_maps.append(dict(
            x=np.ascontiguousarray(xg), wq_t=wq_t, wk_t=wk_t, wv_t=wv_t,
            wp_t=wp_t, bq=bq, bk=k_b.astype(np.float32), bp_eff=bp_eff,
            gamma=norm_gamma.astype(np.float32), beta=norm_beta.astype(np.float32),
            bo=bo, ones=ones))
    return in_maps


def kernel(**inputs):
    x = np.asarray(inputs["x"], np.float32)
    B, _, H, W = x.shape
    n = H * W
    chunks = 8 // B
    nq = n // chunks
    in_maps = make_in_maps(
        x, np.asarray(inputs["q_w"]), np.asarray(inputs["q_b"]),
        np.asarray(inputs["k_w"]), np.asarray(inputs["k_b"]),
        np.asarray(inputs["v_w"]), np.asarray(inputs["v_b"]),
        np.asarray(inputs["proj_w"]), np.asarray(inputs["proj_b"]),
        np.asarray(inputs["norm_gamma"]), np.asarray(inputs["norm_beta"]))
    nc = _get_nc(n, nq)
    res = bass_utils.run_bass_kernel_spmd(nc, in_maps, core_ids=list(range(8)))
    y = np.empty((B, C, n), np.float32)
    for g in range(8):
        b, qc = divmod(g, chunks)
        y[b][:, qc * nq:(qc + 1) * nq] = res.results[g]["y"]
    return y.reshape(B, C, H, W)


# revision 24
# speedup vs baseline: 264.7152x; 2.9946x over previous
"""Self-contained Trainium2 Bass kernel for nn_AttnBlock (VAE-style attention).

Reference computation (per batch b):
  hn = GroupNorm32(x)                      # [C, N], stats per group of 16 chans
  q/k/v = W @ hn + b                       # 1x1 convs, C=512
  attn = softmax(q^T k / sqrt(C), axis=j)  # N=4096 spatial positions
  out  = x + Wp @ (v @ attn^T) + bp

Sharding: 8 cores = 2 batches x 4 query chunks of 1024. Each core receives
its batch's full image ROLLED so its local 1024 query columns come first,
making the SPMD program identical on every core (key order under softmax is
permutation invariant). GroupNorm + K/V are computed over the full image on
each core; Q/proj/residual only for the local chunk.

Layout strategy (c = channel, j = key pos, i = query pos):
  hn   [c-part, n]     -> K tiles  [c-part, j]   (lhsT for scores)
                       -> vT tiles [j-part, c]   (lhsT for AV)
  scoresT = K^T Q      [j-part, i] via matmul(lhsT=k, rhs=q)
  E = exp(scoresT)     [j-part, i] (ScalarE, direct from PSUM)
  outU = vT^T @ E      [c-part, i] accumulated in PSUM over j
  den  = colsum(E)     via DVE accumulation + one all-ones matmul broadcast
  y = Wp @ (outU * recip_den) + (Wp@bv + bp) + x_local

All big matmuls use float32r (full PE rate at free dim >= 256, ~fp32 precision).
"""

import numpy as np

import concourse.bass as bass
import concourse.mybir as mybir
from concourse import bacc
import concourse.tile as tile
from concourse import bass_utils

P = 128          # partitions
C = 512          # channels
CS = C // P      # channel slabs (4)
G = 32           # groups
GS = C // G      # channels per group (16)
EPS = 1e-6
F32 = mybir.dt.float32
F32R = mybir.dt.float32r
AL = mybir.AluOpType
AF = mybir.ActivationFunctionType

N_FULL = 4096    # spatial positions (64*64)
NQ = 1024        # local query chunk per core
JT = 512         # j-tile (keys per outer iteration)
IT = 512         # i-tile (queries per scores matmul free dim)


def build_nc(n=N_FULL, nq=NQ, repeat=1):
    """Build the per-core Bass program. All 8 cores run this same program.

    repeat > 1 wraps the whole compute body in a hardware loop — used only
    for benchmarking (amortizes host dispatch overhead for timing).
    """
    njt = n // JT
    nit = nq // IT

    nc = bacc.Bacc("TRN2", target_bir_lowering=False, debug=False)

    x_d = nc.dram_tensor("x", [C, n], F32, kind="ExternalInput")
    wq_d = nc.dram_tensor("wq_t", [C, C], F32, kind="ExternalInput")  # [cin, cout]
    wk_d = nc.dram_tensor("wk_t", [C, C], F32, kind="ExternalInput")
    wv_d = nc.dram_tensor("wv_t", [C, C], F32, kind="ExternalInput")
    wp_d = nc.dram_tensor("wp_t", [C, C], F32, kind="ExternalInput")
    # all small constants packed into one tensor: per partition p the layout
    # is [bq(CS), bk(CS), bp(CS), gamma(CS), beta(CS), bo_row(P), ones_row(P)]
    cpk_d = nc.dram_tensor("cpk", [P, 5 * CS + 2 * P], F32, kind="ExternalInput")
    y_d = nc.dram_tensor("y", [C, nq], F32, kind="ExternalOutput")

    # [C, n] viewed as [P, CS, n]: channel c = slab*128 + partition
    x_t = x_d.rearrange("(o p) n -> p o n", p=P)
    y_t = y_d.rearrange("(o p) n -> p o n", p=P)

    def r2(d):  # [C] -> [P, CS]
        return d.rearrange("(o p) -> p o", p=P)

    def rw(d):  # [C, C] -> [P, CS, C]  (contraction dim on partitions)
        return d.rearrange("(o p) c -> p o c", p=P)

    with tile.TileContext(nc) as tc:
        with (
            tc.tile_pool(name="consts", bufs=1) as consts,
            tc.tile_pool(name="xp", bufs=1) as xp,
            tc.tile_pool(name="persist", bufs=1) as persist,
            tc.tile_pool(name="kv", bufs=2) as kv,
            tc.tile_pool(name="ep", bufs=4) as ep,
            tc.tile_pool(name="xres", bufs=3) as xres,
            tc.tile_pool(name="yp", bufs=2) as yp,
            tc.tile_pool(name="psmm", bufs=3, space="PSUM") as psmm,
            tc.tile_pool(name="psacc", bufs=4, space="PSUM") as psacc,
        ):
            # ---- constants (outside the repeat loop) ----
            wq_sb = consts.tile([P, CS, C], F32R, tag="wq")
            wk_sb = consts.tile([P, CS, C], F32R, tag="wk")
            wv_sb = consts.tile([P, CS, C], F32R, tag="wv")
            wp_sb = consts.tile([P, CS, C], F32R, tag="wp")
            nc.scalar.dma_start(out=wq_sb, in_=rw(wq_d).bitcast(F32R))
            nc.scalar.dma_start(out=wk_sb, in_=rw(wk_d).bitcast(F32R))
            nc.scalar.dma_start(out=wv_sb, in_=rw(wv_d).bitcast(F32R))
            nc.scalar.dma_start(out=wp_sb, in_=rw(wp_d).bitcast(F32R))
            cpk_sb = consts.tile([P, 5 * CS + 2 * P], F32, tag="cpk")
            nc.scalar.dma_start(out=cpk_sb, in_=cpk_d[:, :])
            bq_sb = cpk_sb[:, 0 * CS:1 * CS]
            bk_sb = cpk_sb[:, 1 * CS:2 * CS]
            bp_sb = cpk_sb[:, 2 * CS:3 * CS]
            gam_sb = cpk_sb[:, 3 * CS:4 * CS]
            bet_sb = cpk_sb[:, 4 * CS:5 * CS]
            bo_sb = cpk_sb[:, 5 * CS:5 * CS + P]
            on_sb = cpk_sb[:, 5 * CS + P:5 * CS + 2 * P]
            eps_sb = consts.tile([P, 1], F32, tag="eps")
            nc.vector.memset(eps_sb, EPS)

            def body():
                # ---- phase 1: load x, groupnorm per slab (groups of 16
                # channels never cross a 128-channel slab), normalize in
                # place. Everything per-slab so Q/K/V matmuls for slab ks
                # unblock as soon as slab ks is normalized.
                x_sbs = []
                for po in range(CS):
                    xs = xp.tile([P, n], F32, tag=f"x{po}", name=f"x{po}")
                    h = n // 2
                    engs = [nc.sync, nc.gpsimd]
                    engs[0].dma_start(out=xs[:, 0:h].bitcast(F32R),
                                      in_=x_t[:, po, 0:h].bitcast(F32R))
                    engs[1].dma_start(out=xs[:, h:n].bitcast(F32R),
                                      in_=x_t[:, po, h:n].bitcast(F32R))
                    x_sbs.append(xs)
                    nchunk = n // 512
                    stats = consts.tile([P, nchunk, 6], F32, tag=f"st{po}",
                                        name=f"st{po}")
                    xs3 = xs.rearrange("p (s f) -> p s f", f=512)
                    for s in range(nchunk):
                        nc.vector.bn_stats(out=stats[:, s, :], in_=xs3[:, s, :])
                    mv = consts.tile([P, 2], F32, tag=f"mv{po}", name=f"mv{po}")
                    nc.vector.bn_aggr(out=mv, in_=stats)
                    # var -> E[x^2] = mean*mean + var (in place)
                    nc.vector.scalar_tensor_tensor(
                        out=mv[:, 1:2], in0=mv[:, 0:1], scalar=mv[:, 0:1],
                        in1=mv[:, 1:2], op0=AL.mult, op1=AL.add)
                    # group-average within the slab: [P, 2] = BO^T @ mv
                    ps_st = psmm.tile([P, 2], F32, tag="ps_mm", name="ps_st")
                    nc.tensor.matmul(ps_st, bo_sb, mv, start=True, stop=True)
                    mvg = consts.tile([P, 2], F32, tag=f"mvg{po}",
                                      name=f"mvg{po}")
                    nc.vector.tensor_copy(out=mvg, in_=ps_st)
                    gmean = mvg[:, 0:1]   # group E[x] per channel
                    gex2 = mvg[:, 1:2]    # group E[x^2] per channel
                    scl = consts.tile([P, 1], F32, tag=f"scl{po}",
                                      name=f"scl{po}")
                    shf = consts.tile([P, 1], F32, tag=f"shf{po}",
                                      name=f"shf{po}")
                    # scl <- -var = mean^2 - E[x^2]
                    nc.vector.scalar_tensor_tensor(
                        out=scl, in0=gmean, scalar=gmean, in1=gex2,
                        op0=AL.mult, op1=AL.subtract)
                    # sqrt(var + eps) via activation scale=-1
                    nc.scalar.activation(out=scl, in_=scl, func=AF.Sqrt,
                                         bias=eps_sb, scale=-1.0)
                    nc.vector.reciprocal(out=scl, in_=scl)
                    nc.vector.tensor_mul(out=scl, in0=scl,
                                         in1=gam_sb[:, po:po + 1])
                    # shf <- gmean*scl - beta = -(true shift)
                    nc.vector.scalar_tensor_tensor(
                        out=shf, in0=gmean, scalar=scl,
                        in1=bet_sb[:, po:po + 1], op0=AL.mult, op1=AL.subtract)
                    # hn = x*scl - shf
                    nc.vector.tensor_scalar(
                        out=xs.bitcast(F32R), in0=xs,
                        scalar1=scl, scalar2=shf,
                        op0=AL.mult, op1=AL.subtract)

                # ---- phase 2: q = Wq_s @ hn + bq_s for local queries ----
                q_sb = persist.tile([P, CS, nq], F32, tag="q", name="q")
                for cc in range(CS):
                    for it in range(nit):
                        isl = slice(it * IT, (it + 1) * IT)
                        ps = psmm.tile([P, IT], F32, tag="ps_mm", name="ps_q")
                        for ks in range(CS):
                            nc.tensor.matmul(
                                ps,
                                wq_sb[:, ks, cc * P:(cc + 1) * P].bitcast(F32R),
                                x_sbs[ks][:, 0:nq][:, isl].bitcast(F32R),
                                start=(ks == 0), stop=(ks == CS - 1))
                        nc.vector.tensor_scalar_add(
                            out=q_sb[:, cc, isl].bitcast(F32R), in0=ps,
                            scalar1=bq_sb[:, cc:cc + 1])

                # ---- phase 3: stream over j-tiles ----
                outu = persist.tile([P, CS, nq], F32, tag="outu", name="outu")
                dens = [persist.tile([P, IT], F32, tag=f"den{it}", name=f"den{it}")
                        for it in range(nit)]

                for jt in range(njt):
                    jsl = slice(jt * JT, (jt + 1) * JT)
                    # k tile [c-part, j]: 16 matmuls
                    k_sb = kv.tile([P, CS, JT], F32, tag="k", name="k")
                    for cc in range(CS):
                        ps = psmm.tile([P, JT], F32, tag="ps_mm", name="ps_k")
                        for ks in range(CS):
                            nc.tensor.matmul(
                                ps,
                                wk_sb[:, ks, cc * P:(cc + 1) * P].bitcast(F32R),
                                x_sbs[ks][:, jsl].bitcast(F32R),
                                start=(ks == 0), stop=(ks == CS - 1))
                        nc.vector.tensor_scalar_add(
                            out=k_sb[:, cc, :].bitcast(F32R), in0=ps,
                            scalar1=bk_sb[:, cc:cc + 1])
                    # vT tile [j-part, c]: 16 matmuls (v bias folded into bp_eff)
                    vt_sb = kv.tile([P, JT // P, C], F32, tag="vt", name="vt")
                    for jc in range(JT // P):
                        jcs = slice(jt * JT + jc * P, jt * JT + (jc + 1) * P)
                        ps = psmm.tile([P, C], F32, tag="ps_mm", name="ps_v")
                        for ks in range(CS):
                            nc.tensor.matmul(
                                ps,
                                x_sbs[ks][:, jcs].bitcast(F32R),
                                wv_sb[:, ks, :].bitcast(F32R),
                                start=(ks == 0), stop=(ks == CS - 1))
                        nc.scalar.copy(out=vt_sb[:, jc, :].bitcast(F32R),
                                       in_=ps)

                    for it in range(nit):
                        isl = slice(it * IT, (it + 1) * IT)
                        pos = [psacc.tile([P, IT], F32, tag="po", name=f"po{cc}")
                               for cc in range(CS)]
                        for jc in range(JT // P):
                            # scoresT chunk [j-part 128, i 512]
                            ps_s = psmm.tile([P, IT], F32, tag="ps_mm", name="ps_s")
                            for ks in range(CS):
                                nc.tensor.matmul(
                                    ps_s,
                                    k_sb[:, ks, jc * P:(jc + 1) * P].bitcast(F32R),
                                    q_sb[:, ks, isl].bitcast(F32R),
                                    start=(ks == 0), stop=(ks == CS - 1))
                            e_sb = ep.tile([P, IT], F32, tag="e", name="e")
                            nc.scalar.activation(out=e_sb.bitcast(F32R), in_=ps_s,
                                                 func=AF.Exp, scale=1.0)
                            # denominator partials (per j-partition)
                            if jt == 0 and jc == 0:
                                nc.gpsimd.tensor_copy(out=dens[it], in_=e_sb)
                            else:
                                nc.gpsimd.tensor_add(out=dens[it], in0=dens[it],
                                                     in1=e_sb)
                            # AV accumulate into psum over this j-tile
                            for cc in range(CS):
                                nc.tensor.matmul(
                                    pos[cc],
                                    vt_sb[:, jc, cc * P:(cc + 1) * P].bitcast(F32R),
                                    e_sb.bitcast(F32R),
                                    start=(jc == 0), stop=(jc == JT // P - 1))
                        # drain psum accumulators into SBUF outU
                        for cc in range(CS):
                            if jt == 0:
                                nc.vector.tensor_copy(out=outu[:, cc, isl],
                                                      in_=pos[cc])
                            else:
                                nc.vector.tensor_add(out=outu[:, cc, isl],
                                                     in0=outu[:, cc, isl],
                                                     in1=pos[cc])

                # ---- phase 4: normalize, proj, residual (it-major so it=0's
                # tail overlaps it=1's attention drain) ----
                recip = persist.tile([P, nq], F32, tag="recip", name="recip")
                for it in range(nit):
                    isl = slice(it * IT, (it + 1) * IT)
                    ps_d = psmm.tile([P, IT], F32, tag="ps_mm", name="ps_d")
                    nc.tensor.matmul(ps_d, on_sb, dens[it], start=True, stop=True)
                    nc.vector.reciprocal(out=recip[:, isl], in_=ps_d)
                    for cc in range(CS):
                        # reuse q_sb (dead after attention) as normalized buffer
                        nc.vector.tensor_tensor(
                            out=q_sb[:, cc, isl].bitcast(F32R),
                            in0=outu[:, cc, isl],
                            in1=recip[:, isl], op=AL.mult)
                    for cc in range(CS):
                        ps = psmm.tile([P, IT], F32, tag="ps_mm", name="ps_p")
                        for ks in range(CS):
                            nc.tensor.matmul(
                                ps,
                                wp_sb[:, ks, cc * P:(cc + 1) * P].bitcast(F32R),
                                q_sb[:, ks, isl].bitcast(F32R),
                                start=(ks == 0), stop=(ks == CS - 1))
                        xr = xres.tile([P, IT], F32, tag="xr", name="xr")
                        nc.sync.dma_start(out=xr, in_=x_t[:, cc, isl])
                        yt = yp.tile([P, IT], F32, tag="yt", name="yt")
                        nc.vector.scalar_tensor_tensor(
                            out=yt, in0=ps, scalar=bp_sb[:, cc:cc + 1], in1=xr,
                            op0=AL.add, op1=AL.add)
                        nc.sync.dma_start(out=y_t[:, cc, isl], in_=yt)

            if repeat == 1:
                body()
            else:
                with tc.For_i(0, repeat, 1):
                    body()

    nc.compile()
    return nc


_NC_CACHE = {}


def _get_nc(n=N_FULL, nq=NQ, repeat=1):
    key = (n, nq, repeat)
    if key not in _NC_CACHE:
        _NC_CACHE[key] = build_nc(n, nq, repeat)
    return _NC_CACHE[key]


def make_in_maps(x, q_w, q_b, k_w, k_b, v_w, v_b, proj_w, proj_b,
                 norm_gamma, norm_beta, n_cores=8):
    """Build per-core input dicts from the full problem inputs."""
    B = x.shape[0]
    n = x.shape[2] * x.shape[3]
    xf = np.ascontiguousarray(x.reshape(B, C, n).astype(np.float32))
    scale = np.float32(C) ** np.float32(-0.5)
    wq_t = np.ascontiguousarray((q_w * scale).T.astype(np.float32))
    wk_t = np.ascontiguousarray(k_w.T.astype(np.float32))
    wv_t = np.ascontiguousarray(v_w.T.astype(np.float32))
    wp_t = np.ascontiguousarray(proj_w.T.astype(np.float32))
    bq = (q_b * scale).astype(np.float32)
    bp_eff = (proj_w.astype(np.float64) @ v_b.astype(np.float64)
              + proj_b.astype(np.float64)).astype(np.float32)
    # block-diagonal group-averaging matrix: 16x16 blocks of 1/16
    bo = np.zeros((P, P), np.float32)
    for g in range(P // GS):
        bo[g * GS:(g + 1) * GS, g * GS:(g + 1) * GS] = 1.0 / GS
    ones = np.ones((P, P), np.float32)
    def r2h(v):  # [C] -> [P, CS] with c = o*P + p
        return np.ascontiguousarray(v.reshape(CS, P).T.astype(np.float32))
    cpk = np.concatenate(
        [r2h(bq), r2h(k_b.astype(np.float32)), r2h(bp_eff),
         r2h(norm_gamma.astype(np.float32)), r2h(norm_beta.astype(np.float32)),
         bo, ones], axis=1)
    chunks = n_cores // B
    nq = n // chunks
    in_maps = []
    for g in range(n_cores):
        b, qc = divmod(g, chunks)
        xg = np.roll(xf[b], -qc * nq, axis=1)
        in_maps.append(dict(
            x=np.ascontiguousarray(xg), wq_t=wq_t, wk_t=wk_t, wv_t=wv_t,
            wp_t=wp_t, cpk=cpk))
    return in_maps


def kernel(**inputs):
    x = np.asarray(inputs["x"], np.float32)
    B, _, H, W = x.shape
    n = H * W
    chunks = 8 // B
    nq = n // chunks
    in_maps = make_in_maps(
        x, np.asarray(inputs["q_w"]), np.asarray(inputs["q_b"]),
        np.asarray(inputs["k_w"]), np.asarray(inputs["k_b"]),
        np.asarray(inputs["v_w"]), np.asarray(inputs["v_b"]),
        np.asarray(inputs["proj_w"]), np.asarray(inputs["proj_b"]),
        np.asarray(inputs["norm_gamma"]), np.asarray(inputs["norm_beta"]))
    nc = _get_nc(n, nq)
    res = bass_utils.run_bass_kernel_spmd(nc, in_maps, core_ids=list(range(8)))
    y = np.empty((B, C, n), np.float32)
    for g in range(8):
        b, qc = divmod(g, chunks)
        y[b][:, qc * nq:(qc + 1) * nq] = res.results[g]["y"]
    return y.reshape(B, C, H, W)


# revision 34
# speedup vs baseline: 301.3589x; 1.1384x over previous
"""Self-contained Trainium2 Bass kernel for nn_AttnBlock (VAE-style attention).

Reference computation (per batch b):
  hn = GroupNorm32(x)                      # [C, N], stats per group of 16 chans
  q/k/v = W @ hn + b                       # 1x1 convs, C=512
  attn = softmax(q^T k / sqrt(C), axis=j)  # N=4096 spatial positions
  out  = x + Wp @ (v @ attn^T) + bp

Sharding: 8 cores = 2 batches x 4 query chunks of 1024. Each core receives
its batch's full image ROLLED so its local 1024 query columns come first,
making the SPMD program identical on every core (key order under softmax is
permutation invariant). GroupNorm + K/V are computed over the full image on
each core; Q/proj/residual only for the local chunk.

Layout strategy (c = channel, j = key pos, i = query pos):
  hn   [c-part, n]     -> K tiles  [c-part, j]   (lhsT for scores)
                       -> vT tiles [j-part, c]   (lhsT for AV)
  scoresT = K^T Q      [j-part, i] via matmul(lhsT=k, rhs=q)
  E = exp(scoresT)     [j-part, i] (ScalarE, direct from PSUM)
  outU = vT^T @ E      [c-part, i] accumulated in PSUM over j
  den  = colsum(E)     via DVE accumulation + one all-ones matmul broadcast
  y = Wp @ (outU * recip_den) + (Wp@bv + bp) + x_local

All big matmuls use float32r (full PE rate at free dim >= 256, ~fp32 precision).
"""

import numpy as np

import concourse.bass as bass
import concourse.mybir as mybir
from concourse import bacc
import concourse.tile as tile
from concourse import bass_utils

P = 128          # partitions
C = 512          # channels
CS = C // P      # channel slabs (4)
G = 32           # groups
GS = C // G      # channels per group (16)
EPS = 1e-6
F32 = mybir.dt.float32
F32R = mybir.dt.float32r
AL = mybir.AluOpType
AF = mybir.ActivationFunctionType

N_FULL = 4096    # spatial positions (64*64)
NQ = 1024        # local query chunk per core
JT = 512         # j-tile (keys per outer iteration)
IT = 512         # i-tile (queries per scores matmul free dim)


def build_nc(n=N_FULL, nq=NQ, repeat=1, ablate=()):
    """Build the per-core Bass program. All 8 cores run this same program.

    repeat > 1 wraps the whole compute body in a hardware loop — used only
    for benchmarking (amortizes host dispatch overhead for timing).
    """
    njt = n // JT
    nit = nq // IT
    ablate = set(ablate)

    nc = bacc.Bacc("TRN2", target_bir_lowering=False, debug=False)

    x_d = nc.dram_tensor("x", [C, n], F32, kind="ExternalInput")
    # w2 = (k_w^T @ (s*q_w)).T — the fused score projection (see module doc)
    w2_d = nc.dram_tensor("w2_t", [C, C], F32, kind="ExternalInput")
    wv_d = nc.dram_tensor("wv_t", [C, C], F32, kind="ExternalInput")
    wp_d = nc.dram_tensor("wp_t", [C, C], F32, kind="ExternalInput")
    # all small constants packed into one tensor: per partition p the layout
    # is [bq(CS), bk(CS), bp(CS), gamma(CS), beta(CS), bo_row(P), ones_row(P)]
    cpk_d = nc.dram_tensor("cpk", [P, 5 * CS + 2 * P], F32, kind="ExternalInput")
    y_d = nc.dram_tensor("y", [C, nq], F32, kind="ExternalOutput")

    # [C, n] viewed as [P, CS, n]: channel c = slab*128 + partition
    x_t = x_d.rearrange("(o p) n -> p o n", p=P)
    y_t = y_d.rearrange("(o p) n -> p o n", p=P)

    def r2(d):  # [C] -> [P, CS]
        return d.rearrange("(o p) -> p o", p=P)

    def rw(d):  # [C, C] -> [P, CS, C]  (contraction dim on partitions)
        return d.rearrange("(o p) c -> p o c", p=P)

    with tile.TileContext(nc) as tc:
        with (
            tc.tile_pool(name="consts", bufs=1) as consts,
            tc.tile_pool(name="xp", bufs=1) as xp,
            tc.tile_pool(name="persist", bufs=1) as persist,
            tc.tile_pool(name="kv", bufs=2) as kv,
            tc.tile_pool(name="ep", bufs=4) as ep,
            tc.tile_pool(name="xres", bufs=3) as xres,
            tc.tile_pool(name="yp", bufs=2) as yp,
            tc.tile_pool(name="psmm", bufs=3, space="PSUM") as psmm,
            tc.tile_pool(name="psacc", bufs=4, space="PSUM") as psacc,
        ):
            # ---- constants (outside the repeat loop) ----
            w2_sb = consts.tile([P, CS, C], F32R, tag="w2")
            wv_sb = consts.tile([P, CS, C], F32R, tag="wv")
            wp_sb = consts.tile([P, CS, C], F32R, tag="wp")
            nc.scalar.dma_start(out=w2_sb, in_=rw(w2_d).bitcast(F32R))
            nc.scalar.dma_start(out=wv_sb, in_=rw(wv_d).bitcast(F32R))
            nc.scalar.dma_start(out=wp_sb, in_=rw(wp_d).bitcast(F32R))
            cpk_sb = consts.tile([P, 5 * CS + 2 * P], F32, tag="cpk")
            nc.scalar.dma_start(out=cpk_sb, in_=cpk_d[:, :])
            bq_sb = cpk_sb[:, 0 * CS:1 * CS]
            bk_sb = cpk_sb[:, 1 * CS:2 * CS]
            bp_sb = cpk_sb[:, 2 * CS:3 * CS]
            gam_sb = cpk_sb[:, 3 * CS:4 * CS]
            bet_sb = cpk_sb[:, 4 * CS:5 * CS]
            bo_sb = cpk_sb[:, 5 * CS:5 * CS + P]
            on_sb = cpk_sb[:, 5 * CS + P:5 * CS + 2 * P]
            eps_sb = consts.tile([P, 1], F32, tag="eps")
            nc.vector.memset(eps_sb, EPS)

            HN = n // 2   # x slabs live as two half tiles for DMA pipelining

            x_pre = []
            if "nodma" in ablate:
                for po in range(CS):
                    parts = []
                    for hh in range(2):
                        xsh = xp.tile([P, HN], F32, tag=f"x{po}_{hh}",
                                      name=f"xp{po}_{hh}")
                        nc.sync.dma_start(
                            out=xsh.bitcast(F32R),
                            in_=x_t[:, po, hh * HN:(hh + 1) * HN].bitcast(F32R))
                        parts.append(xsh)
                    x_pre.append(parts)

            def body():
                # ---- phase 1: load x, groupnorm per slab (groups of 16
                # channels never cross a 128-channel slab), normalize in
                # place. Everything per-slab so Q/K/V matmuls for slab ks
                # unblock as soon as slab ks is normalized.
                x_sbs = []
                for po in range(CS):
                    if "nodma" in ablate:
                        parts = x_pre[po]
                    else:
                        parts = []
                        engs = [nc.sync, nc.gpsimd]
                        for hh in range(2):
                            xsh = xp.tile([P, HN], F32, tag=f"x{po}_{hh}",
                                          name=f"x{po}_{hh}")
                            engs[hh].dma_start(
                                out=xsh.bitcast(F32R),
                                in_=x_t[:, po, hh * HN:(hh + 1) * HN]
                                .bitcast(F32R))
                            parts.append(xsh)
                    x_sbs.append(parts)
                    if "nogn" in ablate:
                        continue
                    nchunk = n // 512
                    nch = nchunk // 2
                    stats = consts.tile([P, nchunk, 6], F32, tag=f"st{po}",
                                        name=f"st{po}")
                    for hh in range(2):
                        xs3 = parts[hh].rearrange("p (s f) -> p s f", f=512)
                        for s in range(nch):
                            nc.vector.bn_stats(out=stats[:, hh * nch + s, :],
                                               in_=xs3[:, s, :])
                    mv = consts.tile([P, 2], F32, tag=f"mv{po}", name=f"mv{po}")
                    nc.vector.bn_aggr(out=mv, in_=stats)
                    # var -> E[x^2] = mean*mean + var (in place)
                    nc.vector.scalar_tensor_tensor(
                        out=mv[:, 1:2], in0=mv[:, 0:1], scalar=mv[:, 0:1],
                        in1=mv[:, 1:2], op0=AL.mult, op1=AL.add)
                    # group-average within the slab: [P, 2] = BO^T @ mv
                    ps_st = psmm.tile([P, 2], F32, tag="ps_mm", name="ps_st")
                    nc.tensor.matmul(ps_st, bo_sb, mv, start=True, stop=True)
                    mvg = consts.tile([P, 2], F32, tag=f"mvg{po}",
                                      name=f"mvg{po}")
                    nc.vector.tensor_copy(out=mvg, in_=ps_st)
                    gmean = mvg[:, 0:1]   # group E[x] per channel
                    gex2 = mvg[:, 1:2]    # group E[x^2] per channel
                    scl = consts.tile([P, 1], F32, tag=f"scl{po}",
                                      name=f"scl{po}")
                    shf = consts.tile([P, 1], F32, tag=f"shf{po}",
                                      name=f"shf{po}")
                    # scl <- -var = mean^2 - E[x^2]
                    nc.vector.scalar_tensor_tensor(
                        out=scl, in0=gmean, scalar=gmean, in1=gex2,
                        op0=AL.mult, op1=AL.subtract)
                    # sqrt(var + eps) via activation scale=-1
                    nc.scalar.activation(out=scl, in_=scl, func=AF.Sqrt,
                                         bias=eps_sb, scale=-1.0)
                    nc.vector.reciprocal(out=scl, in_=scl)
                    nc.vector.tensor_mul(out=scl, in0=scl,
                                         in1=gam_sb[:, po:po + 1])
                    # shf <- gmean*scl - beta = -(true shift)
                    nc.vector.scalar_tensor_tensor(
                        out=shf, in0=gmean, scalar=scl,
                        in1=bet_sb[:, po:po + 1], op0=AL.mult, op1=AL.subtract)
                    # hn = x*scl - shf
                    if "nonorm" not in ablate:
                        for hh in range(2):
                            nc.vector.tensor_scalar(
                                out=parts[hh].bitcast(F32R), in0=parts[hh],
                                scalar1=scl, scalar2=shf,
                                op0=AL.mult, op1=AL.subtract)

                def xap(ks, lo, hi):
                    # slice of slab ks's hn columns [lo, hi) — must lie
                    # within one half tile
                    hh = lo // HN
                    assert hi <= (hh + 1) * HN
                    return x_sbs[ks][hh][:, lo - hh * HN:hi - hh * HN]

                # ---- phase 2: q2 = W2 @ hn + b2 for local queries.
                # W2 folds Wk^T into the scaled Wq so scores = hn^T q2 and
                # K is never materialized (bk's per-query score offset is
                # softmax-invariant and dropped).
                q_sb = persist.tile([P, CS, nq], F32, tag="q", name="q")
                for cc in range(CS):
                    for it in range(nit):
                        isl = slice(it * IT, (it + 1) * IT)
                        ps = psmm.tile([P, IT], F32, tag="ps_mm", name="ps_q")
                        for ks in range(CS):
                            nc.tensor.matmul(
                                ps,
                                w2_sb[:, ks, cc * P:(cc + 1) * P].bitcast(F32R),
                                xap(ks, it * IT, (it + 1) * IT).bitcast(F32R),
                                start=(ks == 0), stop=(ks == CS - 1))
                        nc.vector.tensor_scalar_add(
                            out=q_sb[:, cc, isl].bitcast(F32R), in0=ps,
                            scalar1=bq_sb[:, cc:cc + 1])

                # ---- phase 3: stream over j-tiles ----
                outu = persist.tile([P, CS, nq], F32, tag="outu", name="outu")
                dens = [persist.tile([P, IT], F32, tag=f"den{it}", name=f"den{it}")
                        for it in range(nit)]

                for jt in range(njt):
                    # vT tile [j-part, c]: 16 matmuls (v bias folded into bp_eff)
                    vt_sb = kv.tile([P, JT // P, C], F32, tag="vt", name="vt")
                    for jc in range(JT // P):
                        j0 = jt * JT + jc * P
                        ps = psmm.tile([P, C], F32, tag="ps_mm", name="ps_v")
                        for ks in range(CS):
                            nc.tensor.matmul(
                                ps,
                                xap(ks, j0, j0 + P).bitcast(F32R),
                                wv_sb[:, ks, :].bitcast(F32R),
                                start=(ks == 0), stop=(ks == CS - 1))
                        nc.scalar.copy(out=vt_sb[:, jc, :].bitcast(F32R),
                                       in_=ps)

                    for it in range(nit):
                        isl = slice(it * IT, (it + 1) * IT)
                        pos = [psacc.tile([P, IT], F32, tag="po", name=f"po{cc}")
                               for cc in range(CS)]
                        for jc in range(JT // P):
                            # scoresT chunk [j-part 128, i 512]
                            ps_s = psmm.tile([P, IT], F32, tag="ps_mm", name="ps_s")
                            j0 = jt * JT + jc * P
                            for ks in range(CS):
                                nc.tensor.matmul(
                                    ps_s,
                                    xap(ks, j0, j0 + P).bitcast(F32R),
                                    q_sb[:, ks, isl].bitcast(F32R),
                                    start=(ks == 0), stop=(ks == CS - 1))
                            e_sb = ep.tile([P, IT], F32, tag="e", name="e")
                            nc.scalar.activation(
                                out=e_sb.bitcast(F32R), in_=ps_s,
                                func=(AF.Copy if "noexp" in ablate else AF.Exp),
                                scale=1.0)
                            # denominator partials (per j-partition)
                            if jt == 0 and jc == 0:
                                nc.gpsimd.tensor_copy(out=dens[it], in_=e_sb)
                            else:
                                nc.gpsimd.tensor_add(out=dens[it], in0=dens[it],
                                                     in1=e_sb)
                            # AV accumulate into psum over this j-tile
                            for cc in range(CS):
                                nc.tensor.matmul(
                                    pos[cc],
                                    vt_sb[:, jc, cc * P:(cc + 1) * P].bitcast(F32R),
                                    e_sb.bitcast(F32R),
                                    start=(jc == 0), stop=(jc == JT // P - 1))
                        # drain psum accumulators into SBUF outU
                        if "nodrain" not in ablate or jt == 0:
                            for cc in range(CS):
                                if jt == 0:
                                    nc.vector.tensor_copy(out=outu[:, cc, isl],
                                                          in_=pos[cc])
                                else:
                                    nc.vector.tensor_add(out=outu[:, cc, isl],
                                                         in0=outu[:, cc, isl],
                                                         in1=pos[cc])

                # ---- phase 4: normalize, proj, residual (it-major so it=0's
                # tail overlaps it=1's attention drain) ----
                recip = persist.tile([P, nq], F32, tag="recip", name="recip")
                for it in range(nit):
                    isl = slice(it * IT, (it + 1) * IT)
                    ps_d = psmm.tile([P, IT], F32, tag="ps_mm", name="ps_d")
                    nc.tensor.matmul(ps_d, on_sb, dens[it], start=True, stop=True)
                    nc.vector.reciprocal(out=recip[:, isl], in_=ps_d)
                    for cc in range(CS):
                        # reuse q_sb (dead after attention) as normalized buffer
                        nc.vector.tensor_tensor(
                            out=q_sb[:, cc, isl].bitcast(F32R),
                            in0=outu[:, cc, isl],
                            in1=recip[:, isl], op=AL.mult)
                    for cc in range(CS):
                        ps = psmm.tile([P, IT], F32, tag="ps_mm", name="ps_p")
                        for ks in range(CS):
                            nc.tensor.matmul(
                                ps,
                                wp_sb[:, ks, cc * P:(cc + 1) * P].bitcast(F32R),
                                q_sb[:, ks, isl].bitcast(F32R),
                                start=(ks == 0), stop=(ks == CS - 1))
                        yt = yp.tile([P, IT], F32, tag="yt", name="yt")
                        if "noxres" in ablate:
                            xr = xap(cc, it * IT, (it + 1) * IT)
                        else:
                            xr = xres.tile([P, IT], F32, tag="xr", name="xr")
                            nc.sync.dma_start(out=xr, in_=x_t[:, cc, isl])
                        nc.vector.scalar_tensor_tensor(
                            out=yt, in0=ps, scalar=bp_sb[:, cc:cc + 1], in1=xr,
                            op0=AL.add, op1=AL.add)
                        nc.sync.dma_start(out=y_t[:, cc, isl], in_=yt)

            if repeat == 1:
                body()
            else:
                with tc.For_i(0, repeat, 1):
                    body()

    nc.compile()
    return nc


_NC_CACHE = {}


def _get_nc(n=N_FULL, nq=NQ, repeat=1, ablate=()):
    key = (n, nq, repeat, tuple(sorted(ablate)))
    if key not in _NC_CACHE:
        _NC_CACHE[key] = build_nc(n, nq, repeat, ablate)
    return _NC_CACHE[key]


def make_in_maps(x, q_w, q_b, k_w, k_b, v_w, v_b, proj_w, proj_b,
                 norm_gamma, norm_beta, n_cores=8):
    """Build per-core input dicts from the full problem inputs."""
    B = x.shape[0]
    n = x.shape[2] * x.shape[3]
    xf = np.ascontiguousarray(x.reshape(B, C, n).astype(np.float32))
    scale = np.float64(C) ** -0.5
    # fused score projection: scores^T = hn^T @ (W2 hn + b2) (+ const per
    # query, dropped — softmax invariant)
    w2 = k_w.astype(np.float64).T @ (q_w.astype(np.float64) * scale)
    b2 = k_w.astype(np.float64).T @ (q_b.astype(np.float64) * scale)
    w2_t = np.ascontiguousarray(w2.T.astype(np.float32))
    wv_t = np.ascontiguousarray(v_w.T.astype(np.float32))
    wp_t = np.ascontiguousarray(proj_w.T.astype(np.float32))
    bq = b2.astype(np.float32)
    bp_eff = (proj_w.astype(np.float64) @ v_b.astype(np.float64)
              + proj_b.astype(np.float64)).astype(np.float32)
    # block-diagonal group-averaging matrix: 16x16 blocks of 1/16
    bo = np.zeros((P, P), np.float32)
    for g in range(P // GS):
        bo[g * GS:(g + 1) * GS, g * GS:(g + 1) * GS] = 1.0 / GS
    ones = np.ones((P, P), np.float32)
    def r2h(v):  # [C] -> [P, CS] with c = o*P + p
        return np.ascontiguousarray(v.reshape(CS, P).T.astype(np.float32))
    cpk = np.concatenate(
        [r2h(bq), r2h(k_b.astype(np.float32)), r2h(bp_eff),
         r2h(norm_gamma.astype(np.float32)), r2h(norm_beta.astype(np.float32)),
         bo, ones], axis=1)
    chunks = n_cores // B
    nq = n // chunks
    in_maps = []
    for g in range(n_cores):
        b, qc = divmod(g, chunks)
        xg = np.roll(xf[b], -qc * nq, axis=1)
        in_maps.append(dict(
            x=np.ascontiguousarray(xg), w2_t=w2_t, wv_t=wv_t,
            wp_t=wp_t, cpk=cpk))
    return in_maps


def kernel(**inputs):
    x = np.asarray(inputs["x"], np.float32)
    B, _, H, W = x.shape
    n = H * W
    chunks = 8 // B
    nq = n // chunks
    in_maps = make_in_maps(
        x, np.asarray(inputs["q_w"]), np.asarray(inputs["q_b"]),
        np.asarray(inputs["k_w"]), np.asarray(inputs["k_b"]),
        np.asarray(inputs["v_w"]), np.asarray(inputs["v_b"]),
        np.asarray(inputs["proj_w"]), np.asarray(inputs["proj_b"]),
        np.asarray(inputs["norm_gamma"]), np.asarray(inputs["norm_beta"]))
    nc = _get_nc(n, nq)
    res = bass_utils.run_bass_kernel_spmd(nc, in_maps, core_ids=list(range(8)))
    y = np.empty((B, C, n), np.float32)
    for g in range(8):
        b, qc = divmod(g, chunks)
        y[b][:, qc * nq:(qc + 1) * nq] = res.results[g]["y"]
    return y.reshape(B, C, H, W)


# revision 35
# speedup vs baseline: 301.6342x; 1.0009x over previous
"""Self-contained Trainium2 Bass kernel for nn_AttnBlock (VAE-style attention).

Reference computation (per batch b):
  hn = GroupNorm32(x)                      # [C, N], stats per group of 16 chans
  q/k/v = W @ hn + b                       # 1x1 convs, C=512
  attn = softmax(q^T k / sqrt(C), axis=j)  # N=4096 spatial positions
  out  = x + Wp @ (v @ attn^T) + bp

Sharding: 8 cores = 2 batches x 4 query chunks of 1024. Each core receives
its batch's full image ROLLED so its local 1024 query columns come first,
making the SPMD program identical on every core (key order under softmax is
permutation invariant). GroupNorm + K/V are computed over the full image on
each core; Q/proj/residual only for the local chunk.

Layout strategy (c = channel, j = key pos, i = query pos):
  hn   [c-part, n]     -> K tiles  [c-part, j]   (lhsT for scores)
                       -> vT tiles [j-part, c]   (lhsT for AV)
  scoresT = K^T Q      [j-part, i] via matmul(lhsT=k, rhs=q)
  E = exp(scoresT)     [j-part, i] (ScalarE, direct from PSUM)
  outU = vT^T @ E      [c-part, i] accumulated in PSUM over j
  den  = colsum(E)     via DVE accumulation + one all-ones matmul broadcast
  y = Wp @ (outU * recip_den) + (Wp@bv + bp) + x_local

All big matmuls use float32r (full PE rate at free dim >= 256, ~fp32 precision).
"""

import numpy as np

import concourse.bass as bass
import concourse.mybir as mybir
from concourse import bacc
import concourse.tile as tile
from concourse import bass_utils

P = 128          # partitions
C = 512          # channels
CS = C // P      # channel slabs (4)
G = 32           # groups
GS = C // G      # channels per group (16)
EPS = 1e-6
F32 = mybir.dt.float32
F32R = mybir.dt.float32r
AL = mybir.AluOpType
AF = mybir.ActivationFunctionType

N_FULL = 4096    # spatial positions (64*64)
NQ = 1024        # local query chunk per core
JT = 512         # j-tile (keys per outer iteration)
IT = 512         # i-tile (queries per scores matmul free dim)


def build_nc(n=N_FULL, nq=NQ, repeat=1, ablate=()):
    """Build the per-core Bass program. All 8 cores run this same program.

    repeat > 1 wraps the whole compute body in a hardware loop — used only
    for benchmarking (amortizes host dispatch overhead for timing).
    """
    njt = n // JT
    nit = nq // IT
    ablate = set(ablate)

    nc = bacc.Bacc("TRN2", target_bir_lowering=False, debug=False)

    x_d = nc.dram_tensor("x", [C, n], F32, kind="ExternalInput")
    # w2 = (k_w^T @ (s*q_w)).T — the fused score projection (see module doc)
    w2_d = nc.dram_tensor("w2_t", [C, C], F32, kind="ExternalInput")
    # w3 = (proj_w @ v_w).T — value/proj fused (column scaling by 1/den
    # commutes through the left-multiplications)
    w3_d = nc.dram_tensor("w3_t", [C, C], F32, kind="ExternalInput")
    # all small constants packed into one tensor: per partition p the layout
    # is [bq(CS), bk(CS), bp(CS), gamma(CS), beta(CS), bo_row(P), ones_row(P)]
    cpk_d = nc.dram_tensor("cpk", [P, 5 * CS + 2 * P], F32, kind="ExternalInput")
    y_d = nc.dram_tensor("y", [C, nq], F32, kind="ExternalOutput")

    # [C, n] viewed as [P, CS, n]: channel c = slab*128 + partition
    x_t = x_d.rearrange("(o p) n -> p o n", p=P)
    y_t = y_d.rearrange("(o p) n -> p o n", p=P)

    def r2(d):  # [C] -> [P, CS]
        return d.rearrange("(o p) -> p o", p=P)

    def rw(d):  # [C, C] -> [P, CS, C]  (contraction dim on partitions)
        return d.rearrange("(o p) c -> p o c", p=P)

    with tile.TileContext(nc) as tc:
        with (
            tc.tile_pool(name="consts", bufs=1) as consts,
            tc.tile_pool(name="xp", bufs=1) as xp,
            tc.tile_pool(name="persist", bufs=1) as persist,
            tc.tile_pool(name="kv", bufs=2) as kv,
            tc.tile_pool(name="ep", bufs=4) as ep,
            tc.tile_pool(name="xres", bufs=3) as xres,
            tc.tile_pool(name="yp", bufs=2) as yp,
            tc.tile_pool(name="psmm", bufs=3, space="PSUM") as psmm,
            tc.tile_pool(name="psacc", bufs=4, space="PSUM") as psacc,
        ):
            # ---- constants (outside the repeat loop) ----
            w2_sb = consts.tile([P, CS, C], F32R, tag="w2")
            w3_sb = consts.tile([P, CS, C], F32R, tag="w3")
            nc.scalar.dma_start(out=w2_sb, in_=rw(w2_d).bitcast(F32R))
            nc.scalar.dma_start(out=w3_sb, in_=rw(w3_d).bitcast(F32R))
            cpk_sb = consts.tile([P, 5 * CS + 2 * P], F32, tag="cpk")
            nc.scalar.dma_start(out=cpk_sb, in_=cpk_d[:, :])
            bq_sb = cpk_sb[:, 0 * CS:1 * CS]
            bk_sb = cpk_sb[:, 1 * CS:2 * CS]
            bp_sb = cpk_sb[:, 2 * CS:3 * CS]
            gam_sb = cpk_sb[:, 3 * CS:4 * CS]
            bet_sb = cpk_sb[:, 4 * CS:5 * CS]
            bo_sb = cpk_sb[:, 5 * CS:5 * CS + P]
            on_sb = cpk_sb[:, 5 * CS + P:5 * CS + 2 * P]
            eps_sb = consts.tile([P, 1], F32, tag="eps")
            nc.vector.memset(eps_sb, EPS)

            HN = n // 2   # x slabs live as two half tiles for DMA pipelining

            x_pre = []
            if "nodma" in ablate:
                for po in range(CS):
                    parts = []
                    for hh in range(2):
                        xsh = xp.tile([P, HN], F32, tag=f"x{po}_{hh}",
                                      name=f"xp{po}_{hh}")
                        nc.sync.dma_start(
                            out=xsh.bitcast(F32R),
                            in_=x_t[:, po, hh * HN:(hh + 1) * HN].bitcast(F32R))
                        parts.append(xsh)
                    x_pre.append(parts)

            def body():
                # ---- phase 1: load x, groupnorm per slab (groups of 16
                # channels never cross a 128-channel slab), normalize in
                # place. Everything per-slab so Q/K/V matmuls for slab ks
                # unblock as soon as slab ks is normalized.
                x_sbs = []
                for po in range(CS):
                    if "nodma" in ablate:
                        parts = x_pre[po]
                    else:
                        parts = []
                        engs = [nc.sync, nc.gpsimd]
                        for hh in range(2):
                            xsh = xp.tile([P, HN], F32, tag=f"x{po}_{hh}",
                                          name=f"x{po}_{hh}")
                            engs[hh].dma_start(
                                out=xsh.bitcast(F32R),
                                in_=x_t[:, po, hh * HN:(hh + 1) * HN]
                                .bitcast(F32R))
                            parts.append(xsh)
                    x_sbs.append(parts)
                    if "nogn" in ablate:
                        continue
                    nchunk = n // 512
                    nch = nchunk // 2
                    stats = consts.tile([P, nchunk, 6], F32, tag=f"st{po}",
                                        name=f"st{po}")
                    for hh in range(2):
                        xs3 = parts[hh].rearrange("p (s f) -> p s f", f=512)
                        for s in range(nch):
                            nc.vector.bn_stats(out=stats[:, hh * nch + s, :],
                                               in_=xs3[:, s, :])
                    mv = consts.tile([P, 2], F32, tag=f"mv{po}", name=f"mv{po}")
                    nc.vector.bn_aggr(out=mv, in_=stats)
                    # var -> E[x^2] = mean*mean + var (in place)
                    nc.vector.scalar_tensor_tensor(
                        out=mv[:, 1:2], in0=mv[:, 0:1], scalar=mv[:, 0:1],
                        in1=mv[:, 1:2], op0=AL.mult, op1=AL.add)
                    # group-average within the slab: [P, 2] = BO^T @ mv
                    ps_st = psmm.tile([P, 2], F32, tag="ps_mm", name="ps_st")
                    nc.tensor.matmul(ps_st, bo_sb, mv, start=True, stop=True)
                    mvg = consts.tile([P, 2], F32, tag=f"mvg{po}",
                                      name=f"mvg{po}")
                    nc.vector.tensor_copy(out=mvg, in_=ps_st)
                    gmean = mvg[:, 0:1]   # group E[x] per channel
                    gex2 = mvg[:, 1:2]    # group E[x^2] per channel
                    scl = consts.tile([P, 1], F32, tag=f"scl{po}",
                                      name=f"scl{po}")
                    shf = consts.tile([P, 1], F32, tag=f"shf{po}",
                                      name=f"shf{po}")
                    # scl <- -var = mean^2 - E[x^2]
                    nc.vector.scalar_tensor_tensor(
                        out=scl, in0=gmean, scalar=gmean, in1=gex2,
                        op0=AL.mult, op1=AL.subtract)
                    # sqrt(var + eps) via activation scale=-1
                    nc.scalar.activation(out=scl, in_=scl, func=AF.Sqrt,
                                         bias=eps_sb, scale=-1.0)
                    nc.vector.reciprocal(out=scl, in_=scl)
                    nc.vector.tensor_mul(out=scl, in0=scl,
                                         in1=gam_sb[:, po:po + 1])
                    # shf <- gmean*scl - beta = -(true shift)
                    nc.vector.scalar_tensor_tensor(
                        out=shf, in0=gmean, scalar=scl,
                        in1=bet_sb[:, po:po + 1], op0=AL.mult, op1=AL.subtract)
                    # hn = x*scl - shf
                    if "nonorm" not in ablate:
                        for hh in range(2):
                            nc.vector.tensor_scalar(
                                out=parts[hh].bitcast(F32R), in0=parts[hh],
                                scalar1=scl, scalar2=shf,
                                op0=AL.mult, op1=AL.subtract)

                def xap(ks, lo, hi):
                    # slice of slab ks's hn columns [lo, hi) — must lie
                    # within one half tile
                    hh = lo // HN
                    assert hi <= (hh + 1) * HN
                    return x_sbs[ks][hh][:, lo - hh * HN:hi - hh * HN]

                # ---- phase 2: q2 = W2 @ hn + b2 for local queries.
                # W2 folds Wk^T into the scaled Wq so scores = hn^T q2 and
                # K is never materialized (bk's per-query score offset is
                # softmax-invariant and dropped).
                q_sb = persist.tile([P, CS, nq], F32, tag="q", name="q")
                for cc in range(CS):
                    for it in range(nit):
                        isl = slice(it * IT, (it + 1) * IT)
                        ps = psmm.tile([P, IT], F32, tag="ps_mm", name="ps_q")
                        for ks in range(CS):
                            nc.tensor.matmul(
                                ps,
                                w2_sb[:, ks, cc * P:(cc + 1) * P].bitcast(F32R),
                                xap(ks, it * IT, (it + 1) * IT).bitcast(F32R),
                                start=(ks == 0), stop=(ks == CS - 1))
                        nc.vector.tensor_scalar_add(
                            out=q_sb[:, cc, isl].bitcast(F32R), in0=ps,
                            scalar1=bq_sb[:, cc:cc + 1])

                # ---- phase 3: stream over j-tiles ----
                outu = persist.tile([P, CS, nq], F32, tag="outu", name="outu")
                dens = [persist.tile([P, IT], F32, tag=f"den{it}", name=f"den{it}")
                        for it in range(nit)]

                for jt in range(njt):
                    # vT tile [j-part, c]: 16 matmuls (v bias folded into bp_eff)
                    vt_sb = kv.tile([P, JT // P, C], F32, tag="vt", name="vt")
                    for jc in range(JT // P):
                        j0 = jt * JT + jc * P
                        ps = psmm.tile([P, C], F32, tag="ps_mm", name="ps_v")
                        for ks in range(CS):
                            nc.tensor.matmul(
                                ps,
                                xap(ks, j0, j0 + P).bitcast(F32R),
                                w3_sb[:, ks, :].bitcast(F32R),
                                start=(ks == 0), stop=(ks == CS - 1))
                        nc.scalar.copy(out=vt_sb[:, jc, :].bitcast(F32R),
                                       in_=ps)

                    for it in range(nit):
                        isl = slice(it * IT, (it + 1) * IT)
                        pos = [psacc.tile([P, IT], F32, tag="po", name=f"po{cc}")
                               for cc in range(CS)]
                        for jc in range(JT // P):
                            # scoresT chunk [j-part 128, i 512]
                            ps_s = psmm.tile([P, IT], F32, tag="ps_mm", name="ps_s")
                            j0 = jt * JT + jc * P
                            for ks in range(CS):
                                nc.tensor.matmul(
                                    ps_s,
                                    xap(ks, j0, j0 + P).bitcast(F32R),
                                    q_sb[:, ks, isl].bitcast(F32R),
                                    start=(ks == 0), stop=(ks == CS - 1))
                            e_sb = ep.tile([P, IT], F32, tag="e", name="e")
                            nc.scalar.activation(
                                out=e_sb.bitcast(F32R), in_=ps_s,
                                func=(AF.Copy if "noexp" in ablate else AF.Exp),
                                scale=1.0)
                            # denominator partials (per j-partition)
                            if jt == 0 and jc == 0:
                                nc.gpsimd.tensor_copy(out=dens[it], in_=e_sb)
                            else:
                                nc.gpsimd.tensor_add(out=dens[it], in0=dens[it],
                                                     in1=e_sb)
                            # AV accumulate into psum over this j-tile
                            for cc in range(CS):
                                nc.tensor.matmul(
                                    pos[cc],
                                    vt_sb[:, jc, cc * P:(cc + 1) * P].bitcast(F32R),
                                    e_sb.bitcast(F32R),
                                    start=(jc == 0), stop=(jc == JT // P - 1))
                        # drain psum accumulators into SBUF outU
                        if "nodrain" not in ablate or jt == 0:
                            for cc in range(CS):
                                if jt == 0:
                                    nc.vector.tensor_copy(out=outu[:, cc, isl],
                                                          in_=pos[cc])
                                else:
                                    nc.vector.tensor_add(out=outu[:, cc, isl],
                                                         in0=outu[:, cc, isl],
                                                         in1=pos[cc])

                # ---- phase 4: y = outu*recip + bp_eff + x (proj is folded
                # into the vT3 tiles; per-query 1/den scaling commutes) ----
                recip = persist.tile([P, nq], F32, tag="recip", name="recip")
                for it in range(nit):
                    isl = slice(it * IT, (it + 1) * IT)
                    ps_d = psmm.tile([P, IT], F32, tag="ps_mm", name="ps_d")
                    nc.tensor.matmul(ps_d, on_sb, dens[it], start=True, stop=True)
                    nc.vector.reciprocal(out=recip[:, isl], in_=ps_d)
                    for cc in range(CS):
                        yt = yp.tile([P, IT], F32, tag="yt", name="yt")
                        xr = xres.tile([P, IT], F32, tag="xr", name="xr")
                        nc.sync.dma_start(out=xr, in_=x_t[:, cc, isl])
                        nc.vector.tensor_tensor(
                            out=yt, in0=outu[:, cc, isl],
                            in1=recip[:, isl], op=AL.mult)
                        nc.vector.scalar_tensor_tensor(
                            out=yt, in0=yt, scalar=bp_sb[:, cc:cc + 1], in1=xr,
                            op0=AL.add, op1=AL.add)
                        nc.sync.dma_start(out=y_t[:, cc, isl], in_=yt)

            if repeat == 1:
                body()
            else:
                with tc.For_i(0, repeat, 1):
                    body()

    nc.compile()
    return nc


_NC_CACHE = {}


def _get_nc(n=N_FULL, nq=NQ, repeat=1, ablate=()):
    key = (n, nq, repeat, tuple(sorted(ablate)))
    if key not in _NC_CACHE:
        _NC_CACHE[key] = build_nc(n, nq, repeat, ablate)
    return _NC_CACHE[key]


def make_in_maps(x, q_w, q_b, k_w, k_b, v_w, v_b, proj_w, proj_b,
                 norm_gamma, norm_beta, n_cores=8):
    """Build per-core input dicts from the full problem inputs."""
    B = x.shape[0]
    n = x.shape[2] * x.shape[3]
    xf = np.ascontiguousarray(x.reshape(B, C, n).astype(np.float32))
    scale = np.float64(C) ** -0.5
    # fused score projection: scores^T = hn^T @ (W2 hn + b2) (+ const per
    # query, dropped — softmax invariant)
    w2 = k_w.astype(np.float64).T @ (q_w.astype(np.float64) * scale)
    b2 = k_w.astype(np.float64).T @ (q_b.astype(np.float64) * scale)
    w2_t = np.ascontiguousarray(w2.T.astype(np.float32))
    w3 = proj_w.astype(np.float64) @ v_w.astype(np.float64)
    w3_t = np.ascontiguousarray(w3.T.astype(np.float32))
    bq = b2.astype(np.float32)
    bp_eff = (proj_w.astype(np.float64) @ v_b.astype(np.float64)
              + proj_b.astype(np.float64)).astype(np.float32)
    # block-diagonal group-averaging matrix: 16x16 blocks of 1/16
    bo = np.zeros((P, P), np.float32)
    for g in range(P // GS):
        bo[g * GS:(g + 1) * GS, g * GS:(g + 1) * GS] = 1.0 / GS
    ones = np.ones((P, P), np.float32)
    def r2h(v):  # [C] -> [P, CS] with c = o*P + p
        return np.ascontiguousarray(v.reshape(CS, P).T.astype(np.float32))
    cpk = np.concatenate(
        [r2h(bq), r2h(k_b.astype(np.float32)), r2h(bp_eff),
         r2h(norm_gamma.astype(np.float32)), r2h(norm_beta.astype(np.float32)),
         bo, ones], axis=1)
    chunks = n_cores // B
    nq = n // chunks
    in_maps = []
    for g in range(n_cores):
        b, qc = divmod(g, chunks)
        xg = np.roll(xf[b], -qc * nq, axis=1)
        in_maps.append(dict(
            x=np.ascontiguousarray(xg), w2_t=w2_t, w3_t=w3_t, cpk=cpk))
    return in_maps


def kernel(**inputs):
    x = np.asarray(inputs["x"], np.float32)
    B, _, H, W = x.shape
    n = H * W
    chunks = 8 // B
    nq = n // chunks
    in_maps = make_in_maps(
        x, np.asarray(inputs["q_w"]), np.asarray(inputs["q_b"]),
        np.asarray(inputs["k_w"]), np.asarray(inputs["k_b"]),
        np.asarray(inputs["v_w"]), np.asarray(inputs["v_b"]),
        np.asarray(inputs["proj_w"]), np.asarray(inputs["proj_b"]),
        np.asarray(inputs["norm_gamma"]), np.asarray(inputs["norm_beta"]))
    nc = _get_nc(n, nq)
    res = bass_utils.run_bass_kernel_spmd(nc, in_maps, core_ids=list(range(8)))
    y = np.empty((B, C, n), np.float32)
    for g in range(8):
        b, qc = divmod(g, chunks)
        y[b][:, qc * nq:(qc + 1) * nq] = res.results[g]["y"]
    return y.reshape(B, C, H, W)
